# revision 56
# baseline (speedup 1.0000x reference)
"""Causal sparse (sliding-window) attention for Trainium2, 8 NeuronCores.

Sharding: tensor-parallel over heads (16 heads -> 2 per core).  Each core
computes the qkv projection for its 2 heads (w_qkv column-parallel), windowed
causal attention, and a partial output projection (w_out row-parallel).
The host sums the 8 partial outputs.

v2: everything streams in bf16 (tolerance is 2e-2; bf16 keeps the PE at
1 cyc/row even for <256 moving dims, halves DMA traffic and doubles DVE
throughput).  Attention pipeline is head-merged: the two heads' score
blocks land in one [128, 2, 512] psum pair so exp/mask run as single
wide instructions.  Output is written bf16 and summed on host.

Layout strategy (everything lives transposed so the PE contracts naturally):
  xT [D, L] streamed per 512-column chunk (bf16)
  qT/kT/vT [hd (2 heads packed on partitions), L] from the QKV matmuls
  RoPE: rotate-half is a [128x128] permutation matmul; combine on DVE
  scoresT [k, (h, q)] computed directly (k as lhsT, q as rhs)
  softmax: exp only (scores are small; no max subtraction), masks are
  multiplicative 0/1 on the two partial [128, 2, 128] blocks per query tile
  AV: v augmented with a ones-column -> denominator lands in the psum,
  normalization fused into the psum->sbuf copy
  out projection: ctxT [128, L] directly as lhsT, w_out rows as rhs
"""
import numpy as np

import concourse.bacc as bacc
import concourse.tile as tile
import concourse.mybir as mybir
from concourse.bass_utils import run_bass_kernel_spmd

F32 = mybir.dt.float32
BF16 = mybir.dt.bfloat16

D = 1024
L = 4096
HD = 64
N_CORES = 8
WINDOW = 512
ROPE_BASE = 10000.0
NSB = L // 512          # superblocks of 512 queries
NQB = L // 128          # 128-query blocks


def _attn_plan(sb):
    """Per-superblock key-block plan: (abs key block, lo, hi, diag_qi, far_qi).
    lo/hi bound the valid query blocks (in 0..4) for that key block; diag/far
    mark which query block needs the triangular partial mask."""
    if sb == 0:
        return [(kb, kb, 4, kb, None) for kb in (3, 0, 1, 2)]
    plan = []
    for ki in (0, 4, 1, 5, 2, 7, 3, 6):   # small spans at both ends: the
        # exp of plan[i] must finish before score matmul i+2 can reuse its
        # psum slot, so lead (and trail, for the next superblock's warmup)
        # with short blocks.
        plan.append((sb * 4 - 4 + ki, max(0, ki - 4), min(3, ki) + 1,
                     ki - 4 if ki >= 4 else None, ki if ki <= 3 else None))
    return plan


_TAGS = {}


def _tag(ret, label):
    try:
        _TAGS[ret.ins.name] = label
    except Exception:
        pass
    return ret


def _build_nc(phases=("qkv", "attn", "out"), iters=1):
    _TAGS.clear()
    nc = bacc.Bacc(None, target_bir_lowering=False)

    xT = nc.dram_tensor("xT", [D, L], BF16, kind="ExternalInput")
    wl = nc.dram_tensor("wl", [D, 384], BF16, kind="ExternalInput")
    wo = nc.dram_tensor("wo", [128, D], BF16, kind="ExternalInput")
    p2 = nc.dram_tensor("p2", [128, 128], BF16, kind="ExternalInput")
    cs = nc.dram_tensor("cs", [128, L], BF16, kind="ExternalInput")
    sn = nc.dram_tensor("sn", [128, L], BF16, kind="ExternalInput")
    md = nc.dram_tensor("md", [128, 256], BF16, kind="ExternalInput")
    mf = nc.dram_tensor("mf", [128, 256], BF16, kind="ExternalInput")
    ident = nc.dram_tensor("ident", [128, 128], BF16, kind="ExternalInput")
    onesd = nc.dram_tensor("onesd", [128, 32], BF16, kind="ExternalInput")
    po = nc.dram_tensor("po", [L, D], BF16, kind="ExternalOutput")

    xT3 = xT.rearrange("(ko ki) l -> ki ko l", ki=128)   # [128, 8, L]
    wl3 = wl.rearrange("(ko ki) m -> ki ko m", ki=128)   # [128, 8, 384]

    with tile.TileContext(nc) as tc:
        with tc.tile_pool(name="singles", bufs=1) as singles, \
             tc.tile_pool(name="work", bufs=2) as work, \
             tc.tile_pool(name="ptp", bufs=6) as ptp, \
             tc.tile_pool(name="outp", bufs=4) as outp, \
             tc.tile_pool(name="ps", bufs=2, space="PSUM") as ps:

            w_sb = singles.tile([128, 8, 384], BF16)
            # split so the first matmuls' weights land quickly
            nc.sync.dma_start(w_sb[:, 0:2], wl3[:, 0:2])
            p2_sb = singles.tile([128, 128], BF16)
            id_sb = singles.tile([128, 128], BF16)
            wo_sb = singles.tile([128, D], BF16)
            cs_sb = singles.tile([128, L], BF16)
            sn_sb = singles.tile([128, L], BF16)
            md_sb = singles.tile([128, 2, 128], BF16)
            mf_sb = singles.tile([128, 2, 128], BF16)

            qrot_sb = singles.tile([128, L], BF16)
            krot_sb = singles.tile([128, L], BF16)
            ctxT_sb = singles.tile([128, L], BF16)
            # v natural layout per 128-key block: [h, v(64) | 1]
            v_sb = singles.tile([128, NQB, 2, 65], BF16)

            # only the chunk-0 slice of the rope tables blocks early work;
            # the rest rides in with the other constants after chunk 0.
            # p2/id are read by chunk 0 itself, so they must be emitted
            # before it — on the ACT queue to stay clear of the x stream.
            nc.scalar.dma_start(p2_sb[:], p2[:])
            nc.scalar.dma_start(id_sb[:], ident[:])
            nc.gpsimd.dma_start(cs_sb[:, 0:512], cs[:, 0:512])
            nc.gpsimd.dma_start(sn_sb[:, 0:512], sn[:, 0:512])

            def emit_const_dmas():
                # first reads happen at attention(0)/outproj(0), a full
                # n-step after emission
                nc.gpsimd.dma_start(cs_sb[:, 512:], cs[:, 512:])
                nc.gpsimd.dma_start(sn_sb[:, 512:], sn[:, 512:])
                nc.gpsimd.dma_start(wo_sb[:], wo[:])
                nc.gpsimd.dma_start(md_sb[:], md.rearrange("p (h c) -> p h c", h=2))
                nc.gpsimd.dma_start(mf_sb[:], mf.rearrange("p (h c) -> p h c", h=2))
                nc.gpsimd.dma_start(v_sb[:, :, 0, 64:65], onesd[:, :, None])
                nc.gpsimd.dma_start(v_sb[:, :, 1, 64:65], onesd[:, :, None])

            xt_tiles = {}

            def emit_xt_dma(n):
                span = slice(n * 512, (n + 1) * 512)
                xt = work.tile([128, 8, 512], BF16, tag="xt")
                if n == 0:
                    nc.sync.dma_start(xt[:, 0:2], xT3[:, 0:2, span])
                    nc.sync.dma_start(w_sb[:, 2:8], wl3[:, 2:8])
                    nc.sync.dma_start(xt[:, 2:8], xT3[:, 2:8, span])
                else:
                    nc.sync.dma_start(xt[:], xT3[:, :, span])
                xt_tiles[n] = xt

            def emit_qkv_chunk(n):
                span = slice(n * 512, (n + 1) * 512)
                xt = xt_tiles.pop(n)

                # all three projections back-to-back on PE, copies chase on
                # ACT, then the rotate-half matmuls (their ACT inputs are
                # ready by then), then the v transposes.
                prj = []
                for m in range(3):
                    psq = ps.tile([128, 512], F32, tag="mm")
                    for k8 in range(8):
                        _tag(nc.tensor.matmul(
                            psq[:], w_sb[:, k8, m * 128:(m + 1) * 128],
                            xt[:, k8, :], start=(k8 == 0), stop=(k8 == 7)),
                            f"qkvmm n{n} m{m} k{k8}")
                    raw = work.tile([128, 512], BF16,
                                    tag="qkraw" if m < 2 else "vraw")
                    if m < 2:
                        nc.scalar.copy(raw[:], psq[:])
                    else:
                        nc.vector.tensor_copy(raw[:], psq[:])
                    prj.append((psq, raw))

                # v transposes first: their DVE copies drain the psum slots
                # fast, whereas the rot matmuls' slots are held until the DVE
                # rope chain reads them.
                vraw = prj[2][1]
                for j in range(4):
                    blk = n * 4 + j
                    tp = ps.tile([128, 128], BF16, tag="mm",
                                 padded_shape=[128, 1024])
                    _tag(nc.tensor.transpose(tp[:], vraw[:, j * 128:(j + 1) * 128],
                                        id_sb[:]), f"vtp n{n} j{j}")
                    nc.vector.tensor_copy(
                        v_sb[:, blk, :, 0:64],
                        tp.rearrange("p (h c) -> p h c", h=2))

                for m, dst in ((0, qrot_sb), (1, krot_sb)):
                    raw = prj[m][1]
                    psr = ps.tile([128, 512], F32, tag="mm")
                    _tag(nc.tensor.matmul(psr[:], p2_sb[:], raw[:],
                                     start=True, stop=True), f"rotmm n{n} m{m}")
                    qc = work.tile([128, 512], BF16, tag="qc")
                    nc.vector.tensor_tensor(qc[:], raw[:], cs_sb[:, span],
                                            mybir.AluOpType.mult)
                    qs = work.tile([128, 512], BF16, tag="qs")
                    nc.vector.tensor_tensor(qs[:], psr[:], sn_sb[:, span],
                                            mybir.AluOpType.mult)
                    nc.vector.tensor_tensor(dst[:, span], qc[:], qs[:],
                                            mybir.AluOpType.add)

            def emit_attention_sb(sb, fillers=()):
                fillers = list(fillers)
                plan = _attn_plan(sb)
                n_av = len(plan)
                ctxs = [ps.tile([65, 512], F32, tag="ctx", bufs=2,
                                padded_shape=[128, 512], name=f"ctx{h}")
                        for h in range(2)]

                def emit_score(idx, mask_eng):
                    kb, lo, hi, diag_qi, far_qi = plan[idx]
                    cspan = slice(lo * 128, hi * 128)
                    qspan = slice(sb * 512 + lo * 128, sb * 512 + hi * 128)
                    pts_h = []
                    for h in range(2):
                        hp = slice(h * 64, (h + 1) * 64)
                        scp = ps.tile([128, 512], F32, tag="scp", bufs=4,
                                      name="scp")
                        _tag(nc.tensor.matmul(
                            scp[:, cspan],
                            krot_sb[hp, kb * 128:(kb + 1) * 128],
                            qrot_sb[hp, qspan],
                            start=True, stop=True,
                            tile_position=(h * 64, 0)), f"scmm sb{sb} h{h} i{idx}")
                        pt = ptp.tile([128, 512], BF16, tag="pt", name="pt",
                                      bufs=10)
                        nc.scalar.activation(
                            pt[:, cspan], scp[:, cspan],
                            mybir.ActivationFunctionType.Exp, scale=0.125)
                        if far_qi is not None:
                            fsp = slice(far_qi * 128, (far_qi + 1) * 128)
                            mask_eng.tensor_tensor(pt[:, fsp], pt[:, fsp],
                                                   mf_sb[:, h],
                                                   mybir.AluOpType.mult)
                        if diag_qi is not None:
                            dsp = slice(diag_qi * 128, (diag_qi + 1) * 128)
                            mask_eng.tensor_tensor(pt[:, dsp], pt[:, dsp],
                                                   md_sb[:, h],
                                                   mybir.AluOpType.mult)
                        pts_h.append(pt)
                    return pts_h

                # AVs run in a different order than the scores: the first
                # executed AV must be the full-span entry (start=True zeroes
                # only the region it writes), while the scores lead with
                # short spans to soften the scp-recycle warmup.
                full_idx = next(i for i, p in enumerate(plan)
                                if p[1] == 0 and p[2] == 4)
                av_order = [full_idx] + [i for i in range(n_av)
                                         if i != full_idx]

                def emit_av(k, pt_of):
                    idx = av_order[k]
                    kb, lo, hi, _, _ = plan[idx]
                    cspan = slice(lo * 128, hi * 128)
                    for h in range(2):
                        _tag(nc.tensor.matmul(
                            ctxs[h][0:65, cspan],
                            v_sb[:, kb, h, :],
                            pt_of[idx][h][:, cspan],
                            start=(k == 0), stop=(k == n_av - 1),
                            skip_group_check=True), f"avmm sb{sb} h{h} i{idx}")

                # software pipeline: AV lags the score/exp/mask chain by one
                # key block, both heads interleaved, so the PE never waits on
                # the ACT+POOL round trip.
                LAG = 3 if n_av > 4 else 2
                pts = {}
                for idx in range(n_av):
                    pts[idx] = emit_score(idx, nc.vector)
                    if idx >= LAG:
                        emit_av(idx - LAG, pts)
                    # hand-place outproj matmul groups between score steps:
                    # they fill the PE while exp/mask round-trips drain scp
                    if idx % 2 == 1 and fillers:
                        fillers.pop(0)()
                for k in range(n_av - LAG, n_av):
                    emit_av(k, pts)

                # norm: issue per-stage across both heads so the two heads'
                # chains pipeline (DVE is in-order; h1's recip must not sit
                # behind h0's pool broadcast round trip).  Leftover fillers
                # come after so their psum->sbuf copies don't delay the
                # reciprocals in the DVE queue.
                sspan = slice(sb * 512, (sb + 1) * 512)
                # 1/x = exp(-ln x): both live in the same ACT table set, so
                # this replaces the DVE iterative reciprocal (~6 cyc/elem on
                # one lane) with two fast ACT passes and no table switch.
                rts, rbs = [], []
                for h in range(2):
                    lt = work.tile([1, 512], F32, tag="lt")
                    nc.scalar.activation(lt[:], ctxs[h][64:65, :],
                                         mybir.ActivationFunctionType.Ln)
                    rts.append(lt)
                for h in range(2):
                    rt = work.tile([1, 512], F32, tag="rt")
                    nc.scalar.activation(rt[:], rts[h][:],
                                         mybir.ActivationFunctionType.Exp,
                                         scale=-1.0)
                    rts[h] = rt
                for h in range(2):
                    rb = work.tile([64, 512], F32, tag="rb")
                    nc.gpsimd.partition_broadcast(rb[:], rts[h][:])
                    rbs.append(rb)
                for h in range(2):
                    hp = slice(h * 64, (h + 1) * 64)
                    nc.vector.tensor_tensor(ctxT_sb[hp, sspan],
                                            ctxs[h][0:64, :],
                                            rbs[h][:], mybir.AluOpType.mult)
                for f in fillers:
                    f()

            def outproj_t(t, tag="mm"):
                def emit():
                    osb = outp.tile([128, 1024], BF16, tag="ob")
                    if tag == "scp":
                        # final superblock: borrow the (idle) score psum
                        # slots so all four t-groups pipeline at the drain
                        ops = [ps.tile([128, 512], F32, tag="scp", bufs=4,
                                       name="scp") for _ in range(2)]
                    else:
                        ops = None
                    for nn in range(2):
                        op = ops[nn] if ops else ps.tile([128, 512], F32,
                                                         tag="mm")
                        _tag(nc.tensor.matmul(
                            op[:], ctxT_sb[:, t * 128:(t + 1) * 128],
                            wo_sb[:, nn * 512:(nn + 1) * 512],
                            start=True, stop=True), f"outmm t{t} n{nn}")
                        if nn == 0:
                            nc.scalar.copy(osb[:, 0:512], op[:])
                        else:
                            nc.vector.tensor_copy(osb[:, 512:1024], op[:])
                    nc.sync.dma_start(po[t * 128:(t + 1) * 128, :], osb[:])
                return emit

            def emit_body():
                for n in range(NSB + 2):
                    if n == 0 and "qkv" in phases:
                        emit_xt_dma(0)
                    if n + 1 < NSB and "qkv" in phases:
                        emit_xt_dma(n + 1)   # prefetch next chunk's x
                    if n < NSB and "qkv" in phases:
                        emit_qkv_chunk(n)
                    if n == 0:
                        emit_const_dmas()
                    fillers = ()
                    if n >= 2 and "out" in phases:
                        last = (n == NSB + 1)
                        fillers = [outproj_t(t, "scp" if last and ti < 2
                                             else "mm")
                                   for ti, t in enumerate(
                                       range((n - 2) * 4, (n - 2) * 4 + 4))]
                    if 1 <= n <= NSB and "attn" in phases:
                        emit_attention_sb(n - 1, fillers)
                    elif "out" in phases:
                        for f in fillers:
                            f()

            if iters == 1:
                emit_body()
            else:
                with tc.For_i(0, iters, 1):
                    emit_body()
    nc.finalize()
    return nc


def _host_constants():
    import ml_dtypes
    bf16 = ml_dtypes.bfloat16
    # RoPE tables, transposed + duplicated for the two packed head halves
    inv_freq = (1.0 / (ROPE_BASE ** (np.arange(0, HD, 2, dtype=np.float32)
                                     / np.float32(HD)))).astype(np.float32)
    pos = np.arange(L, dtype=np.float32)
    freqs = pos[:, None] * inv_freq[None, :]            # [L, 32]
    cos = np.repeat(np.cos(freqs), 2, axis=-1).astype(np.float32)  # [L, 64]
    sin = np.repeat(np.sin(freqs), 2, axis=-1).astype(np.float32)
    cs = np.ascontiguousarray(np.vstack([cos.T, cos.T])).astype(bf16)  # [128, L]
    sn = np.ascontiguousarray(np.vstack([sin.T, sin.T])).astype(bf16)

    # rotate-half as a column-space permutation: rh(q) = q @ Pc
    pc = np.zeros((HD, HD), np.float32)
    for m in range(HD // 2):
        pc[2 * m + 1, 2 * m] = -1.0
        pc[2 * m, 2 * m + 1] = 1.0
    p2 = np.zeros((128, 128), np.float32)
    p2[:64, :64] = pc
    p2[64:, 64:] = pc
    p2 = p2.astype(bf16)

    k_idx = np.arange(128)[:, None]
    q_idx = np.arange(128)[None, :]
    md1 = (k_idx <= q_idx).astype(np.float32)   # diag block: valid k <= q
    mf1 = (k_idx > q_idx).astype(np.float32)    # far block: valid k > q
    md = np.ascontiguousarray(np.hstack([md1, md1])).astype(bf16)  # [128, 256]
    mf = np.ascontiguousarray(np.hstack([mf1, mf1])).astype(bf16)
    ident = np.eye(128, dtype=np.float32).astype(bf16)
    onesd = np.ones((128, 32), bf16)
    return cs, sn, p2, md, mf, ident, onesd


_NC_CACHE = {}


def kernel(x, w_qkv, w_out):
    import ml_dtypes
    bf16 = ml_dtypes.bfloat16
    x = np.asarray(x, np.float32)
    w_qkv = np.asarray(w_qkv, np.float32)
    w_out = np.asarray(w_out, np.float32)
    B = x.shape[0]
    assert x.shape == (B, L, D) and B == 1

    if "nc" not in _NC_CACHE:
        _NC_CACHE["nc"] = _build_nc()
    nc = _NC_CACHE["nc"]

    xT = np.ascontiguousarray(x[0].T).astype(bf16)     # [D, L]
    cs, sn, p2, md, mf, ident, onesd = _host_constants()

    in_maps = []
    for c in range(N_CORES):
        h0 = 2 * c
        col = slice(h0 * HD, (h0 + 2) * HD)
        wl = np.ascontiguousarray(np.concatenate(
            [w_qkv[:, 0 * D:1 * D][:, col],
             w_qkv[:, 1 * D:2 * D][:, col],
             w_qkv[:, 2 * D:3 * D][:, col]], axis=1)).astype(bf16)  # [D, 384]
        wo = np.ascontiguousarray(
            w_out[h0 * HD:(h0 + 2) * HD, :]).astype(bf16)  # [128, D]
        in_maps.append({"xT": xT, "wl": wl, "wo": wo, "p2": p2,
                        "cs": cs, "sn": sn, "md": md, "mf": mf,
                        "ident": ident, "onesd": onesd})

    res = run_bass_kernel_spmd(nc, in_maps, core_ids=list(range(N_CORES)))
    out = np.zeros((L, D), np.float64)
    for r in res.results:
        out += r["po"].astype(np.float64)
    return out.astype(np.float32)[None]
